# revision 13
# baseline (speedup 1.0000x reference)
"""VQ codebook kernel v2: fp32r screen + exact fp32 rescore of top-8.

Per m-tile of 128 rows:
  screen: PSUM = ones x (-esq) [fp32r K=1 bias matmul] + x·2e [fp32r matmuls]
          ACT copies PSUM -> d_sb (pure evict, no DVE subtract needed)
  top-8:  vector.max + vector.max_index over d_sb (DVE, the only full scans)
  rescore: gather 8 candidate rows from etab_aug[K,257] (e_k, ||e_k||^2),
          PE-transpose them into an rhs pool [c, 8*128], one exact fp32 matmul
          -> diagonal entries give exact dots; s = 2*dot - esq exact;
          small max/max_index over 8 + iota-select yields the final index.
  outputs: final gather e[idx] -> x_q; PE transposes -> x_q_detach, x_fea.
"""

import os
from contextlib import ExitStack

import numpy as np

import concourse.bacc as bacc
import concourse.bass as bass
import concourse.mybir as mybir
import concourse.tile as tile
from concourse.bass_utils import run_bass_kernel_spmd
from concourse.masks import make_identity

B, C, H, W = 16, 256, 32, 32
K = 8192
NCORES = 8
NB = B // NCORES
HWP = H * W
P = 128
CC = C // P
NSPAN = 8
SPAN = K // NSPAN
J = 6                      # rescore candidates per row
CA = C + 1                 # augmented row: e_k plus ||e_k||^2
F32 = mybir.dt.float32
F32R = mybir.dt.float32r
U32 = mybir.dt.uint32

_CACHE = {}
last_results = None


def _build_nc():
    nc = bacc.Bacc("TRN2", target_bir_lowering=False, debug=False)

    xs = nc.dram_tensor("xs", [NB * C, HWP], F32, kind="ExternalInput")
    xsr = nc.dram_tensor("xsr", [NB * C, HWP], F32R, kind="ExternalInput")
    e2t = nc.dram_tensor("e2t", [C, K], F32R, kind="ExternalInput")
    esqn = nc.dram_tensor("esqn", [1, K], mybir.dt.float16, kind="ExternalInput")
    etaba = nc.dram_tensor("etaba", [K, CA], F32, kind="ExternalInput")

    xq = nc.dram_tensor("xq", [NB * HWP, C], F32, kind="ExternalOutput")
    xqd = nc.dram_tensor("xqd", [NB * C, HWP], F32, kind="ExternalOutput")
    xfea = nc.dram_tensor("xfea", [NB * HWP, C], F32, kind="ExternalOutput")

    xs_ap, xsr_ap, e2t_ap, esqn_ap, etaba_ap = (
        xs.ap(), xsr.ap(), e2t.ap(), esqn.ap(), etaba.ap())
    xq_ap, xqd_ap, xfea_ap = xq.ap(), xqd.ap(), xfea.ap()

    with tile.TileContext(nc) as tc, ExitStack() as ctx:
        const = ctx.enter_context(tc.tile_pool(name="const", bufs=1))
        dpool = ctx.enter_context(tc.tile_pool(name="d", bufs=2))
        psum = ctx.enter_context(tc.tile_pool(name="ps", bufs=2, space="PSUM"))
        tpsum = ctx.enter_context(tc.tile_pool(name="tps", bufs=2, space="PSUM"))
        rpsum = ctx.enter_context(tc.tile_pool(name="rps", bufs=2, space="PSUM"))
        small = ctx.enter_context(tc.tile_pool(name="small", bufs=4))
        gpool = ctx.enter_context(tc.tile_pool(name="g", bufs=1))
        opool = ctx.enter_context(tc.tile_pool(name="o", bufs=2))

        ident = const.tile([P, P], F32)
        make_identity(nc, ident[:])
        # identity blocks side by side: mask for diagonal extraction
        NPJ = 384 // P
        identblk = const.tile([P, NPJ, P], F32)
        for j in range(NPJ):
            nc.scalar.copy(identblk[:, j, :], ident[:])
        # ones rows at each legal base partition; bias matmul runs in fp16
        # (its rounding only perturbs the screen; the exact rescore fixes it)
        ones_r = const.tile([96, P], mybir.dt.float16)
        nc.vector.memset(ones_r[:], 1.0)
        # -esq packed so each 512-group sits at a matmul-legal base
        # partition (0/32/64): group g -> partition 32*(g%3), column g//3
        esqn_sb = const.tile([96, 6, 512], mybir.dt.float16)
        for g in range(K // 512):
            bp = 32 * (g % 3)
            nc.sync.dma_start(esqn_sb[bp:bp + 1, g // 3, :],
                              esqn_ap[:, g * 512:(g + 1) * 512])
        # iota over J slots, fp32 values 0..7, one column each
        iota8 = const.tile([P, 8], F32)
        for j in range(8):
            nc.vector.memset(iota8[:, j:j + 1], float(j))

        e2t_sb = const.tile([P, CC * K], F32R)
        for cc in range(CC):
            nc.sync.dma_start(e2t_sb[:, cc * K:(cc + 1) * K],
                              e2t_ap[cc * P:(cc + 1) * P, :])

        x_sb = const.tile([P, NB * CC * HWP], F32)
        xr_sb = const.tile([P, NB * CC * HWP], F32R)
        for b in range(NB):
            for cc in range(CC):
                sl = slice(b * C + cc * P, b * C + (cc + 1) * P)
                dst = slice((b * CC + cc) * HWP, (b * CC + cc + 1) * HWP)
                nc.sync.dma_start(x_sb[:, dst], xs_ap[sl, :])
                nc.sync.dma_start(xr_sb[:, dst], xsr_ap[sl, :])

        def finish_argmin(st):
            d_sb = st["d"]
            top8 = small.tile([P, 8], F32, tag="top8")
            nc.vector.max(top8[:], d_sb[:])
            idx8 = small.tile([P, 8], U32, tag="idx8")
            nc.vector.max_index(idx8[:], top8[:], d_sb[:])
            st["idx8"] = idx8

        def finish_rescore(st):
            b, m0, idx8 = st["b"], st["m0"], st["idx8"]
            # gather the J candidate rows (+their norms) per x-row
            g_all = gpool.tile([P, J, CA], F32, tag="gall")
            for j in range(J):
                nc.gpsimd.indirect_dma_start(
                    out=g_all[:, j, :], out_offset=None,
                    in_=etaba_ap[:, :],
                    in_offset=bass.IndirectOffsetOnAxis(ap=idx8[:, j:j + 1],
                                                        axis=0))
            # transpose candidates into an rhs pool [c-chunk part, J*P]
            ect = gpool.tile([P, CC, J * P], F32, tag="ect")
            for j in range(J):
                for cc in range(CC):
                    tp = tpsum.tile([P, P], F32, tag="tp")
                    nc.tensor.transpose(
                        tp[:], g_all[:, j, cc * P:(cc + 1) * P], ident[:])
                    nc.scalar.copy(ect[:, cc, j * P:(j + 1) * P], tp[:])
            # exact fp32 dot products: out[m, j*P+mm] (diag mm==m is row m's)
            dot8 = small.tile([P, J], F32, tag="dot8")
            for half in range(2):
                rp = rpsum.tile([P, 384], F32, tag="rp")
                for cc in range(CC):
                    xoff = (b * CC + cc) * HWP + m0
                    nc.tensor.matmul(
                        rp[:],
                        lhsT=x_sb[:, xoff:xoff + P],
                        rhs=ect[:, cc, half * 384:(half + 1) * 384],
                        start=(cc == 0), stop=(cc == CC - 1))
                # diagonal extraction: mask with block-identity, then reduce
                npj = 384 // P
                scr = gpool.tile([P, npj, P], F32, tag="scr")
                nc.vector.tensor_tensor(
                    scr[:].rearrange("p a b -> p (a b)"), rp[:],
                    identblk[:].rearrange("p a b -> p (a b)"),
                    op=mybir.AluOpType.mult)
                nc.vector.tensor_reduce(
                    dot8[:, half * npj:(half + 1) * npj], scr[:],
                    axis=mybir.AxisListType.X, op=mybir.AluOpType.add)
            # exact s = 2*dot - esq
            esq8 = g_all[:, :, C]          # [P, J] strided view
            s8 = small.tile([P, 8], F32, tag="s8")
            nc.vector.memset(s8[:, J:], -3.0e38)
            nc.vector.tensor_scalar(s8[:, :J], dot8[:], 2.0, None,
                                    op0=mybir.AluOpType.mult)
            nc.vector.tensor_tensor(s8[:, :J], s8[:, :J], esq8,
                                    op=mybir.AluOpType.subtract)
            # pick best of J, then select its original codebook index
            b8 = small.tile([P, 8], F32, tag="b8")
            nc.vector.max(b8[:], s8[:])
            j8 = small.tile([P, 8], U32, tag="j8")
            nc.vector.max_index(j8[:], b8[:], s8[:])
            jstar = small.tile([P, 1], F32, tag="jstar")
            nc.vector.tensor_copy(jstar[:], j8[:, 0:1])
            mask = small.tile([P, J], F32, tag="mask")
            nc.vector.tensor_scalar(mask[:], iota8[:, :J], jstar[:, 0:1], None,
                                    op0=mybir.AluOpType.is_equal)
            idx8f = small.tile([P, J], F32, tag="idx8f")
            nc.vector.tensor_copy(idx8f[:], idx8[:, :J])
            nc.vector.tensor_tensor(mask[:], mask[:], idx8f[:],
                                    op=mybir.AluOpType.mult)
            fidxf = small.tile([P, 1], F32, tag="fidxf")
            nc.vector.tensor_reduce(fidxf[:], mask[:],
                                    axis=mybir.AxisListType.X,
                                    op=mybir.AluOpType.max)
            fidx = small.tile([P, 1], U32, tag="fidx")
            nc.vector.tensor_copy(fidx[:], fidxf[:])
            st["fidx"] = fidx

        def finish_outputs(st):
            b, m0, fidx = st["b"], st["m0"], st["fidx"]
            g = opool.tile([P, CA], F32, tag="g")
            nc.gpsimd.indirect_dma_start(
                out=g[:], out_offset=None,
                in_=etaba_ap[:, :],
                in_offset=bass.IndirectOffsetOnAxis(ap=fidx[:, 0:1], axis=0))
            row0 = b * HWP + m0
            nc.sync.dma_start(xq_ap[row0:row0 + P, :], g[:, :C])
            for cc in range(CC):
                tq = tpsum.tile([P, P], F32, tag="tp")
                nc.tensor.transpose(tq[:], g[:, cc * P:(cc + 1) * P], ident[:])
                tq_sb = opool.tile([P, P], F32, tag="tqsb")
                nc.scalar.copy(tq_sb[:], tq[:])
                nc.sync.dma_start(
                    xqd_ap[b * C + cc * P: b * C + (cc + 1) * P, m0:m0 + P],
                    tq_sb[:])
                xoff = (b * CC + cc) * HWP + m0
                tf = tpsum.tile([P, P], F32, tag="tp")
                nc.tensor.transpose(tf[:], x_sb[:, xoff:xoff + P], ident[:])
                tf_sb = opool.tile([P, P], F32, tag="tfsb")
                nc.scalar.copy(tf_sb[:], tf[:])
                nc.sync.dma_start(
                    xfea_ap[row0:row0 + P, cc * P:(cc + 1) * P], tf_sb[:])

        prev = None
        for _rep in range(int(os.environ.get("VQ_REPEAT", "1"))):
          for b in range(NB):
            for mt in range(HWP // P):
                m0 = mt * P
                d_sb = dpool.tile([P, K], F32, tag="dsb")
                for s in range(NSPAN):
                    ps = psum.tile([P, SPAN], F32, tag="dps")
                    for h in range(SPAN // 512):
                        hs = slice(h * 512, (h + 1) * 512)
                        ks = s * SPAN + h * 512
                        # bias: ones^T x (-esq) fills the accumulation group
                        grp = ks // 512
                        bp = 32 * (grp % 3)
                        nc.tensor.matmul(
                            ps[:, hs],
                            lhsT=ones_r[bp:bp + 1, :],
                            rhs=esqn_sb[bp:bp + 1, grp // 3, :],
                            start=True, stop=False)
                        for cc in range(CC):
                            xoff = (b * CC + cc) * HWP + m0
                            nc.tensor.matmul(
                                ps[:, hs],
                                lhsT=xr_sb[:, xoff:xoff + P],
                                rhs=e2t_sb[:, cc * K + ks:cc * K + ks + 512],
                                start=False, stop=(cc == CC - 1))
                    # evict (pure copy, bias already applied in PSUM)
                    nc.scalar.copy(d_sb[:, s * SPAN:(s + 1) * SPAN], ps[:])
                    if prev is not None:
                        if s == 1:
                            finish_argmin(prev)
                        elif s == 3:
                            finish_rescore(prev)
                        elif s == 6:
                            finish_outputs(prev)
                            prev = None
                prev = {"d": d_sb, "b": b, "m0": m0}
        finish_argmin(prev)
        finish_rescore(prev)
        finish_outputs(prev)
    nc.compile()
    return nc


def kernel(x, embedding_weight):
    global last_results
    x = np.ascontiguousarray(np.asarray(x), dtype=np.float32)
    e = np.ascontiguousarray(np.asarray(embedding_weight), dtype=np.float32)
    assert x.shape == (B, C, H, W) and e.shape == (K, C)

    e2t = np.ascontiguousarray(2.0 * e.T)
    esq = np.sum(e * e, axis=1, dtype=np.float32)
    esqn = np.ascontiguousarray((-esq[None, :]).astype(np.float16))
    etaba = np.ascontiguousarray(np.concatenate([e, esq[:, None]], axis=1))

    if "nc" not in _CACHE:
        _CACHE["nc"] = _build_nc()
    nc = _CACHE["nc"]

    in_maps = []
    for c in range(NCORES):
        xsh = np.ascontiguousarray(x[c * NB:(c + 1) * NB].reshape(NB * C, HWP))
        in_maps.append({"xs": xsh, "xsr": xsh, "e2t": e2t, "esqn": esqn,
                        "etaba": etaba})

    trace = bool(int(os.environ.get("VQ_TRACE", "0")))
    br = run_bass_kernel_spmd(nc, in_maps, core_ids=list(range(NCORES)),
                              trace=trace)
    last_results = br

    xq_o = np.empty((B, H, W, C), np.float32)
    xqd_o = np.empty((B, C, H, W), np.float32)
    xfea_o = np.empty((B, H, W, C), np.float32)
    for c in range(NCORES):
        r = br.results[c]
        sl = slice(c * NB, (c + 1) * NB)
        xq_o[sl] = r["xq"].reshape(NB, H, W, C)
        xqd_o[sl] = r["xqd"].reshape(NB, C, H, W)
        xfea_o[sl] = r["xfea"].reshape(NB, H, W, C)
    return (xqd_o, xq_o, xfea_o)
